# revision 1
# baseline (speedup 1.0000x reference)
"""MultiHeadRichAttention Trainium2 Bass kernel (8-core data parallel).

Math (per batch b, folding done host-side):
  x = [keys, q, keys*q, keys-q] @ W1f  ==  [keysT; (keys*q)T] @ W12 + C[b]
      where W12 = [W1A+W1D; W1C], C = q @ (W1B - W1D) + b1
  H1 = prelu(mm1 + C, a1); H2 = prelu(H1 @ W2bd + b2, a2)
  scores = H2 @ W3bd    (b3 dropped: softmax-invariant)
  w = softmax_masked(scores); wbar = mean_h w
  out = wbar @ (keys @ Wo + bo)   (bo fold exact: sum_s wbar = 1)

Per-core layout: 256 batches = 128 pairs = 32 bank-groups of 4 pairs (8 b).
Scores col-packed into one PSUM bank per group via tile_position; softmax
batched on [128, 400] tiles; head-mean via strided-AP reduce after PE
transpose; final matmul per batch with [s,1] stationaries.
"""
import numpy as np
import ml_dtypes

import concourse.bass as bass
import concourse.bacc as bacc
import concourse.tile as tile
from concourse import mybir
from concourse.bass_utils import run_bass_kernel_spmd

F32 = mybir.dt.float32
BF16 = mybir.dt.bfloat16
AX = mybir.AxisListType
ALU = mybir.AluOpType
ACTF = mybir.ActivationFunctionType

NCORES = 8
B, S, D, H = 2048, 200, 64, 4
H1N, H2N = 64, 32
BL = B // NCORES          # 256 batches per core
NPAIR = BL // 2           # 128
NGRP = NPAIR // 4         # 32 bank-groups (8 batches each)
SC0, SC1 = 128, S - 128   # s-chunks 128 + 72

bf16 = ml_dtypes.bfloat16


def _register_prelu_op():
    import concourse.dve_ops as dve_ops
    from concourse.dve_ops import DveOp, OPS, CUSTOM_DVE_SPECS, _SUB_OPCODE_FOR_NAME
    from concourse.dve_spec import Spec, Src0, C0, maxx, lower
    from concourse.dve_uop import DveOpSpec

    if "PRELU_ANT" in CUSTOM_DVE_SPECS:
        return next(op for op in OPS if op.name == "PRELU_ANT")
    spec = Spec(
        body=maxx(Src0, Src0 * C0),
        reference=lambda in0, in1, s0, s1, imm2: np.maximum(
            in0.astype(np.float32), in0.astype(np.float32) * s0
        ),
    )
    row = dve_ops._CUSTOM_DVE_ROW_BASE + len(OPS)
    shas = {}
    for ver in ("v3", "v4"):
        try:
            tmp = DveOpSpec(name="PRELU_ANT", opcode=row,
                            uops=lower(spec, ver=ver), rd1_en=False)
            shas[ver] = tmp.sha(ver)
        except Exception:
            pass
    op = DveOp("PRELU_ANT", spec, subdim=False, uops_sha=shas)
    OPS.append(op)
    CUSTOM_DVE_SPECS["PRELU_ANT"] = spec
    _SUB_OPCODE_FOR_NAME["PRELU_ANT"] = row
    return op


def build_nc(n_groups=NGRP):
    """Build the per-core Bass program (same program on all 8 cores)."""
    from contextlib import ExitStack

    PRELU = _register_prelu_op()
    nc = bacc.Bacc("TRN2", target_bir_lowering=False, debug=False,
                   num_devices=NCORES)

    x2t_d = nc.dram_tensor("x2t", [BL, 128, S], BF16, kind="ExternalInput").ap()
    knw_d = nc.dram_tensor("knw", [BL, S, D], BF16, kind="ExternalInput").ap()
    mp_d = nc.dram_tensor("maskpack", [NGRP, 128, 2 * S], F32,
                          kind="ExternalInput").ap()
    # packed constants: cb (bf16): w12 0:256 | cn0 256:512 | cn1 512:768 |
    # w2_0 768:896 | w2_1 896:1024 | w3pad 1024:1056 | eye 1056:1184
    cb_d = nc.dram_tensor("cb", [128, 1184], BF16, kind="ExternalInput").ap()
    # pv (f32): 0 a1c0 | 1 zeros | 2 a1c1 | 3 a2 | 4 b2 | 5 zeros
    pv_d = nc.dram_tensor("pv", [128, 6], F32, kind="ExternalInput").ap()
    out_d = nc.dram_tensor("out", [BL, D], F32, kind="ExternalOutput").ap()

    with tile.TileContext(nc) as tc, ExitStack() as ctx:
        const = ctx.enter_context(tc.tile_pool(name="const", bufs=1))
        x2p = ctx.enter_context(tc.tile_pool(name="x2p", bufs=3))
        h1p = ctx.enter_context(tc.tile_pool(name="h1p", bufs=3))
        h2p = ctx.enter_context(tc.tile_pool(name="h2p", bufs=3))
        smp = ctx.enter_context(tc.tile_pool(name="smp", bufs=2))
        knwp = ctx.enter_context(tc.tile_pool(name="knwp", bufs=2))
        mpp = ctx.enter_context(tc.tile_pool(name="mpp", bufs=2))
        wbtp = ctx.enter_context(tc.tile_pool(name="wbtp", bufs=2))
        pfcp = ctx.enter_context(tc.tile_pool(name="pfcp", bufs=2))
        p1p = ctx.enter_context(tc.tile_pool(name="p1p", bufs=3, space="PSUM"))
        p2p = ctx.enter_context(tc.tile_pool(name="p2p", bufs=2, space="PSUM"))
        scbp = ctx.enter_context(tc.tile_pool(name="scbp", bufs=1, space="PSUM"))
        wtp = ctx.enter_context(tc.tile_pool(name="wtp", bufs=1, space="PSUM"))
        pfp = ctx.enter_context(tc.tile_pool(name="pfp", bufs=1, space="PSUM"))

        cb_t = const.tile([128, 1184], BF16)
        pv_t = const.tile([128, 6], F32)
        nc.sync.dma_start(out=cb_t, in_=cb_d)
        nc.sync.dma_start(out=pv_t, in_=pv_d)
        w12_t = cb_t[:, 0:256]
        cn_t = [cb_t[:, 256:512], cb_t[:, 512:768]]
        w2_t = [cb_t[:, 768:896], cb_t[:, 896:1024]]
        w3_t = cb_t[:, 1024:1056]
        eye_t = cb_t[:, 1056:1184]
        a1_t = [pv_t[:, 0:1], pv_t[:, 2:3]]
        zb_t = pv_t[:, 1:2]
        a2_t = pv_t[:, 3:4]
        b2_t = pv_t[:, 4:5]

        for g in range(n_groups):
            G = g // (NGRP // 2)
            b0 = 8 * g
            # group DMAs: kNW s-chunks [s, (bg e)], packed mask
            knw0 = knwp.tile([SC0, 8 * D], BF16, tag="knw0", name="knw0")
            knw1 = knwp.tile([SC1, 8 * D], BF16, tag="knw1", name="knw1")
            # src AP [s, (b, e)]: strides in elements (b: S*D, s: D, e: 1)
            nc.sync.dma_start(
                out=knw0,
                in_=bass.AP(knw_d.tensor, b0 * S * D,
                            [[D, SC0], [S * D, 8], [1, D]]))
            nc.sync.dma_start(
                out=knw1,
                in_=bass.AP(knw_d.tensor, b0 * S * D + SC0 * D,
                            [[D, SC1], [S * D, 8], [1, D]]))
            mp_t = mpp.tile([128, 2 * S], F32, tag="mp", name="mp")
            nc.sync.dma_start(out=mp_t, in_=mp_d[g])

            scb_t = scbp.tile([128, 2 * S], F32, tag="scb", name="scb")
            h2_ts = []
            for lp4 in range(4):
                i = 4 * g + lp4
                lp = i % 64
                x2_t = x2p.tile([128, 2 * S], BF16, tag="x2", name="x2")
                nc.sync.dma_start(
                    out=x2_t.rearrange("p (t s) -> p t s", t=2),
                    in_=x2t_d[2 * i:2 * i + 2].rearrange("b p s -> p b s"))

                h1_ts = []
                for c in range(2):
                    p1_t = p1p.tile([128, 2 * S], F32, tag="p1", name="p1")
                    nc.tensor.matmul(p1_t, w12_t[:, 128 * c:128 * (c + 1)],
                                     x2_t, start=True, stop=False)
                    oh = eye_t[:, 2 * lp:2 * lp + 2]
                    oh_ap = bass.AP(oh.tensor, oh.offset,
                                    [oh.ap[0], oh.ap[1], [0, S]])
                    nc.tensor.matmul(p1_t, cn_t[G][:, 128 * c:128 * (c + 1)],
                                     oh_ap, start=False, stop=True)
                    h1_t = h1p.tile([128, 2 * S], BF16, tag=f"h1_{c}",
                                    name=f"h1_{c}")
                    if c == 0:
                        nc.scalar.activation(h1_t, p1_t, ACTF.Prelu,
                                             bias=zb_t, alpha=a1_t[0])
                    else:
                        nc.vector._custom_dve(PRELU, out=h1_t, in0=p1_t,
                                              s0=a1_t[1])
                    h1_ts.append(h1_t)

                p2_t = p2p.tile([128, 2 * S], F32, tag="p2", name="p2")
                nc.tensor.matmul(p2_t, w2_t[0], h1_ts[0], start=True, stop=False)
                nc.tensor.matmul(p2_t, w2_t[1], h1_ts[1], start=False, stop=True)
                h2_t = h2p.tile([128, 2 * S], BF16, tag="h2", name="h2")
                nc.scalar.activation(h2_t, p2_t, ACTF.Prelu,
                                     bias=b2_t, alpha=a2_t)
                h2_ts.append(h2_t)
                nc.tensor.matmul(scb_t[32 * lp4:32 * (lp4 + 1), :], w3_t, h2_t,
                                 start=True, stop=True,
                                 tile_position=(0, 32 * lp4))

            # ---- softmax over the group's score bank ----
            sm_t = smp.tile([128, 2 * S], F32, tag="sm", name="sm")
            nc.vector.tensor_add(sm_t, scb_t, mp_t)
            e_t = smp.tile([128, 2 * S], BF16, tag="e", name="e")
            nc.scalar.activation(e_t, sm_t, ACTF.Exp, bias=zb_t)
            ss_t = wbtp.tile([128, 2], F32, tag="ss", name="ss")
            nc.vector.tensor_reduce(
                out=ss_t, in_=e_t.rearrange("p (t s) -> p t s", t=2),
                axis=AX.X, op=ALU.add, opt_input=False)
            ss4_t = wbtp.tile([128, 2], F32, tag="ss4", name="ss4")
            nc.vector.tensor_scalar_max(ss4_t, ss_t, 1e-30)
            ss5_t = wbtp.tile([128, 2], F32, tag="ss5", name="ss5")
            nc.vector.tensor_scalar_mul(ss5_t, ss4_t, 4.0)
            r4_t = wbtp.tile([128, 2], F32, tag="r4", name="r4")
            nc.vector.reciprocal(r4_t, ss5_t)
            w_t = smp.tile([128, 2 * S], BF16, tag="w", name="w")
            nc.vector.tensor_scalar_mul(w_t[:, 0:S], e_t[:, 0:S], r4_t[:, 0:1])
            nc.vector.tensor_scalar_mul(w_t[:, S:2 * S], e_t[:, S:2 * S],
                                        r4_t[:, 1:2])

            # ---- transpose + head-sum -> wbarT [s, (bb j)] ----
            wt_t = wtp.tile([128, 512], BF16, tag="wt", name="wt")
            cuts = [(0, 0, SC0), (1, SC0, SC1), (2, S, SC0), (3, S + SC0, SC1)]
            for r, (ci, c0, clen) in enumerate(cuts):
                nc.tensor.transpose(wt_t[0:clen, 128 * ci:128 * ci + 128],
                                    w_t[:, c0:c0 + clen], eye_t)
            wbt0 = wbtp.tile([SC0, 8], BF16, tag="wbt0", name="wbt0")
            wbt1 = wbtp.tile([SC1, 8], BF16, tag="wbt1", name="wbt1")
            with nc.allow_low_precision(reason="4-elt head-sum bf16"):
                for bb in range(2):
                    for sc, (wbt, clen) in enumerate([(wbt0, SC0), (wbt1, SC1)]):
                        reg = wt_t[0:clen, 128 * (2 * bb + sc):128 * (2 * bb + sc) + 128]
                        ap3 = bass.AP(reg.tensor, reg.offset,
                                      [reg.ap[0], [32, 4], [1, 4]])
                        nc.vector.tensor_reduce(
                            out=wbt[:, 4 * bb:4 * bb + 4], in_=ap3,
                            axis=AX.X, op=ALU.add, opt_input=False)

            # ---- final: out rows via per-batch [s,1] stationaries ----
            pf_t = pfp.tile([1, 512], F32, tag="pf", name="pf")
            for bg in range(8):
                j, bb = bg // 2, bg % 2
                nc.tensor.matmul(pf_t[0:1, D * bg:D * (bg + 1)],
                                 wbt0[:, 4 * bb + j:4 * bb + j + 1],
                                 knw0[:, D * bg:D * (bg + 1)],
                                 start=True, stop=False)
                nc.tensor.matmul(pf_t[0:1, D * bg:D * (bg + 1)],
                                 wbt1[:, 4 * bb + j:4 * bb + j + 1],
                                 knw1[:, D * bg:D * (bg + 1)],
                                 start=False, stop=True)
            # always DVE: keeps the next group's final-matmul waits on 2 procs
            pfc_t = pfcp.tile([1, 512], F32, tag="pfc", name="pfc")
            nc.vector.tensor_copy(pfc_t, pf_t)
            nc.sync.dma_start(
                out=out_d[b0:b0 + 8, :].rearrange("b d -> (b d)")[None, :],
                in_=pfc_t)
    nc.compile()
    return nc


def prep_inputs(query, keys, keys_mask, W1, b1, a1, W2, b2, a2, W3, b3, Wo, bo):
    """Host-side folding; returns per-core in_maps."""
    q = np.asarray(query, np.float32)
    keys = np.asarray(keys, np.float32)
    mask = np.asarray(keys_mask)
    W1 = np.asarray(W1, np.float32)
    W1f = np.transpose(W1, (1, 0, 2)).reshape(4 * D, H * H1N)
    W1A, W1B, W1C, W1D = (W1f[0:D], W1f[D:2 * D], W1f[2 * D:3 * D],
                          W1f[3 * D:4 * D])
    W12 = np.concatenate([W1A + W1D, W1C], 0).astype(bf16)            # [128,256]
    b1f = np.asarray(b1, np.float32).reshape(H * H1N)
    C = (q @ (W1B - W1D) + b1f).astype(bf16)                          # [B,256]
    W2bd = np.zeros((H * H1N, H * H2N), np.float32)
    W2a = np.asarray(W2, np.float32)
    for h in range(H):
        W2bd[H1N * h:H1N * (h + 1), H2N * h:H2N * (h + 1)] = W2a[h]
    W2bd = W2bd.astype(bf16)
    W3pad = np.zeros((H * H2N, 32), np.float32)
    W3a = np.asarray(W3, np.float32)
    for h in range(H):
        W3pad[H2N * h:H2N * (h + 1), h] = W3a[h]
    W3pad = W3pad.astype(bf16)

    a1f = np.asarray(a1, np.float32)
    a2f = np.asarray(a2, np.float32)
    pv = np.zeros((128, 6), np.float32)
    pv[:, 0] = np.repeat(a1f[0:2], H1N)
    pv[:, 2] = np.repeat(a1f[2:4], H1N)
    pv[:, 3] = np.repeat(a2f, H2N)
    pv[:, 4] = np.asarray(b2, np.float32).reshape(128)

    kT = np.ascontiguousarray(keys.transpose(0, 2, 1))
    kqT = np.ascontiguousarray((keys * q[:, None, :]).transpose(0, 2, 1))
    X2T = np.concatenate([kT, kqT], 1).astype(bf16)                   # [B,128,S]
    kNW = ((keys.reshape(-1, D) @ np.asarray(Wo, np.float32)
            + np.asarray(bo, np.float32)).reshape(B, S, D)).astype(bf16)

    m4 = (np.asarray(mask, np.float32) - 1.0) * 1e30                  # [B,S]
    mp = np.full((NCORES, NGRP, 4, 32, 2, S), -1e30, np.float32)
    mp[:, :, :, 0:4, :, :] = m4.reshape(NCORES, NGRP, 4, 2, S)[:, :, :, None, :, :]
    mp = np.ascontiguousarray(mp.reshape(NCORES, NGRP, 128, 2 * S))

    eye = np.eye(128, dtype=np.float32).astype(bf16)
    Cn = np.ascontiguousarray(C.reshape(NCORES, 2, 128, H * H1N))

    in_maps = []
    for cix in range(NCORES):
        sl = slice(cix * BL, (cix + 1) * BL)
        cb = np.concatenate([
            W12, Cn[cix, 0], Cn[cix, 1], W2bd[0:128],
            W2bd[128:256], W3pad, eye], axis=1).astype(bf16)
        in_maps.append({
            "x2t": X2T[sl], "knw": kNW[sl], "maskpack": mp[cix],
            "cb": np.ascontiguousarray(cb), "pv": pv,
        })
    return in_maps


_NC_CACHE = {}


def get_nc():
    if "nc" not in _NC_CACHE:
        _NC_CACHE["nc"] = build_nc()
    return _NC_CACHE["nc"]


def kernel(**inputs) -> np.ndarray:
    in_maps = prep_inputs(**inputs)
    nc = get_nc()
    res = run_bass_kernel_spmd(nc, in_maps, core_ids=list(range(NCORES)))
    return np.concatenate([r["out"] for r in res.results], 0)

